# revision 21
# baseline (speedup 1.0000x reference)
"""Bahdanau attention kernel for Trainium2, 8 NeuronCores, data-parallel over batch.

Reference computation (per batch b):
    hq = query @ Wq_w.T + Wq_b          # [Q, A]
    hy = y @ Wy_w.T + Wy_b              # [Y, A]
    scores[q, y] = v_w . tanh(hq[q] + hy[y]) + v_b   # [Q, Y]
    att = softmax(scores, axis=y)       # [Q, Y]
    sim[q] = max_y scores[q, y]         # [1, Q]

Shapes: B=8, Q=256, Y=512, D=1024, A=256.

Kernel strategy (one batch per core):
  - Keep the A dim on SBUF partitions (2 tiles of 128).
  - hqT [a, q], hyT [a, y] computed via PE matmuls on transposed operands
    (PE transpose-mode for query/y/W transposes), float32r matmul dtype.
  - Per q: S[a, y] = hyT[a, y] + hqT'[a, q] via DVE tensor_scalar_add
    (per-partition scalar); batches of 16 q's are tanh'd in a single
    big ACT instruction (in-place, bf16).
  - Score dot: PE matmul with a sliding-window stationary ZV (v at
    column 127) so out[q, y] lands directly in a [128, 512] PSUM tile.
  - Softmax: DVE reduce_max -> ACT Exp(bias=-max, accum_out=sum) ->
    DVE reciprocal -> scale.  sim = max + v_b.
"""

import numpy as np

B, Q, Y, D, A = 8, 256, 512, 1024, 256
KT = D // 128   # k tiles in contraction dim
AT = A // 128   # a tiles
QB = Q // 128   # q blocks
YTILES = Y // 128
QTILES = Q // 128
QC = 32         # max q's per ACT chunk (S tile sizing)
NCH = 128 // QC

_cached = None


def _build():
    import concourse.bass as bass
    import concourse.tile as tile
    from concourse import bacc, mybir
    from concourse import masks

    f32 = mybir.dt.float32
    f32r = mybir.dt.float32r
    bf16 = mybir.dt.bfloat16
    ts = bass.ts
    AF = mybir.ActivationFunctionType

    nc = bacc.Bacc("TRN2", target_bir_lowering=False, debug=False)

    qT_ext = nc.dram_tensor("qTv", [D, Q], bf16, kind="ExternalInput")
    yT_ext = nc.dram_tensor("yTv", [D, Y], bf16, kind="ExternalInput")
    wqT_ext = nc.dram_tensor("wqTv", [D, A], bf16, kind="ExternalInput")
    wyT_ext = nc.dram_tensor("wyTv", [D, A], bf16, kind="ExternalInput")
    wqb_ext = nc.dram_tensor("Wq_b", [A], f32, kind="ExternalInput")
    wyb_ext = nc.dram_tensor("Wy_b", [A], f32, kind="ExternalInput")
    v_ext = nc.dram_tensor("v_w", [1, A], f32, kind="ExternalInput")
    vb_ext = nc.dram_tensor("v_b", [1], f32, kind="ExternalInput")
    att_ext = nc.dram_tensor("att", [Q, Y], f32, kind="ExternalOutput")
    sim_ext = nc.dram_tensor("sim", [1, Q], f32, kind="ExternalOutput")

    with tile.TileContext(nc) as tc:
        from contextlib import ExitStack
        ctx = ExitStack()
        with ctx:
            consts = ctx.enter_context(tc.tile_pool(name="consts", bufs=1))
            tr_sb = ctx.enter_context(tc.tile_pool(name="tr_sb", bufs=1))
            s_pool = ctx.enter_context(tc.tile_pool(name="s", bufs=4))
            soft_pool = ctx.enter_context(tc.tile_pool(name="soft", bufs=2))
            small = ctx.enter_context(tc.tile_pool(name="small", bufs=2))
            psum_proj = ctx.enter_context(
                tc.tile_pool(name="ps_proj", bufs=1, space="PSUM"))
            psum_sc = ctx.enter_context(
                tc.tile_pool(name="ps_sc", bufs=2, space="PSUM"))

            # ---- constants ----
            ones_row = consts.tile([1, 128], f32)
            nc.gpsimd.memset(ones_row[:], 1.0)

            # per-partition vectors (DMAs issued later, after the big
            # input loads, so they don't delay them on the sync queue)
            bq_sb = [consts.tile([128, 1], f32, name=f"bq{t}") for t in range(AT)]
            by_sb = [consts.tile([128, 1], f32, name=f"by{t}") for t in range(AT)]
            v_sb = [consts.tile([128, 1], f32, name=f"v{t}") for t in range(AT)]
            vb_sb = consts.tile([1, 1], f32)
            cb = [consts.tile([128, 1], f32, name=f"cb{t}") for t in range(AT)]
            vb_bc = consts.tile([128, 1], f32)
            zv = [consts.tile([128, 256], bf16, name=f"zv{t}") for t in range(AT)]

            def small_consts():
                for t in range(AT):
                    nc.sync.dma_start(bq_sb[t][:], wqb_ext.ap()[ts(t, 128)].unsqueeze(1))
                    nc.sync.dma_start(by_sb[t][:], wyb_ext.ap()[ts(t, 128)].unsqueeze(1))
                    nc.sync.dma_start(v_sb[t][:], v_ext.ap()[0, ts(t, 128)].unsqueeze(1))
                nc.sync.dma_start(vb_sb[:], vb_ext.ap().unsqueeze(0))
                for t in range(AT):
                    nc.vector.tensor_add(cb[t][:], bq_sb[t][:], by_sb[t][:])
                ps_vb = psum_proj.tile([128, 1], f32, tag="hy")
                nc.tensor.matmul(ps_vb[:], ones_row[:], vb_sb[:], start=True, stop=True)
                nc.vector.tensor_copy(vb_bc[:], ps_vb[:])
                for t in range(AT):
                    nc.gpsimd.memset(zv[t][:], 0.0)
                    nc.vector.tensor_copy(zv[t][:, 127:128], v_sb[t][:])

            # ---- load transposed bf16 inputs (host-marshalled) ----
            ident = consts.tile([128, 128], bf16)
            masks.make_identity(nc, ident[:])
            ident_f32 = consts.tile([128, 128], f32)
            masks.make_identity(nc, ident_f32[:])

            # SBUF layout: block k at columns [k*M, (k+1)*M)
            qT = tr_sb.tile([128, KT * Q], bf16, name="qT")
            yT = tr_sb.tile([128, KT * Y], bf16, name="yT")
            wqT = tr_sb.tile([128, KT * A], bf16, name="wqT")
            wyT = tr_sb.tile([128, KT * A], bf16, name="wyT")

            def load_T(ext, dst, nsplit=1):
                # DRAM [D, M] -> SBUF [128, KT*M]: k-tile k of rows
                # [k*128,(k+1)*128) lands at columns [k*M,(k+1)*M).
                # nsplit>1 issues per-k-group DMAs so consumers (the
                # projection matmul for k) can start before the whole
                # tensor has arrived.
                dst3 = dst[:].rearrange("p (k m) -> p k m", k=KT)
                src3 = ext.ap().rearrange("(k p) m -> p k m", p=128)
                g = KT // nsplit
                for s in range(nsplit):
                    nc.sync.dma_start(dst3[:, s * g:(s + 1) * g, :],
                                      src3[:, s * g:(s + 1) * g, :])

            # PE warm-up: ~20 dummy matmuls flip the HAM clock gate to
            # 2.4 GHz before the projection chains run; consumed via a
            # DRAM scratch store so DCE keeps them.
            warm_dram = nc.dram_tensor("warm_scratch", [128, 128], f32)
            ps_warm = psum_sc.tile([128, 128], f32, tag="warm", bufs=1)
            NWARM = 44
            for w in range(NWARM):
                nc.tensor.matmul(ps_warm[:], ident[:], ident[:],
                                 start=(w == 0), stop=(w == NWARM - 1))
            warm_sb = small.tile([128, 128], f32, tag="warm")
            nc.vector.tensor_copy(warm_sb[:], ps_warm[:])
            nc.sync.dma_start(warm_dram.ap(), warm_sb[:])

            hyT = [tr_sb.tile([128, Y], bf16, name=f"hyT{t}") for t in range(AT)]
            hqT = [tr_sb.tile([128, Q], f32, name=f"hqT{t}") for t in range(AT)]

            def proj_hy(t):
                ps = psum_proj.tile([128, Y], f32, tag="hy")
                for k in range(KT):
                    nc.tensor.matmul(
                        ps[:], wyT[:, k * A + t * 128:k * A + t * 128 + 128],
                        yT[:, ts(k, Y)],
                        start=(k == 0), stop=(k == KT - 1))
                nc.vector.tensor_copy(hyT[t][:], ps[:])

            def proj_hq(t, h, c0=0, c1=128):
                # one q-column range of half h (separate accumulation group)
                ps = psum_proj.tile([128, 128], f32, tag="hq")
                n = c1 - c0
                for k in range(KT):
                    nc.tensor.matmul(
                        ps[:, 0:n],
                        wqT[:, k * A + t * 128:k * A + t * 128 + 128],
                        qT[:, k * Q + h * 128 + c0:k * Q + h * 128 + c1],
                        start=(k == 0), stop=(k == KT - 1))
                nc.vector.tensor_scalar_add(
                    hqT[t][:, h * 128 + c0:h * 128 + c1], ps[:, 0:n],
                    cb[t][:, 0:1])

            # ---- main loop pieces ----
            def sweep(qb, t, ps_scores, chunks, first_sweep, last_sweep):
                # one (q-block, a-tile) pass: S = hy + hq[q] (DVE), tanh
                # (ACT, in-place), score-dot MMs into ps_scores.
                q0 = 0
                for ci, qc in enumerate(chunks):
                    S = s_pool.tile([128, QC * Y], bf16, tag="S")
                    for j in range(qc):
                        q = qb * 128 + q0 + j
                        nc.vector.tensor_scalar_add(
                            S[:, ts(j, Y)], hyT[t][:], hqT[t][:, q:q + 1])
                    nc.scalar.activation(S[:, 0:qc * Y], S[:, 0:qc * Y], AF.Tanh)
                    for j in range(qc):
                        ql = q0 + j
                        first = (first_sweep and ci == 0 and j == 0)
                        last = (last_sweep and ci == len(chunks) - 1
                                and j == qc - 1)
                        nc.tensor.matmul(
                            ps_scores[:],
                            zv[t][:, 127 - ql:255 - ql],
                            S[:, ts(j, Y)],
                            start=first, stop=last)
                    q0 += qc

            def softmax_block(qb, ps_scores, split_out):
                mx = small.tile([128, 1], f32, tag="mx")
                nc.vector.reduce_max(mx[:], ps_scores[:], axis=mybir.AxisListType.X)
                sim_sb = small.tile([128, 1], f32, tag="sim")
                nc.vector.tensor_add(sim_sb[:], mx[:], vb_bc[:])
                ps_simT = psum_proj.tile([1, 128], f32, tag="hq", name=f"psimT{qb}")
                nc.tensor.transpose(ps_simT[:], sim_sb[:], ident_f32[:])
                sim_row = small.tile([1, 128], f32, tag="simrow")
                nc.vector.tensor_copy(sim_row[:], ps_simT[:])
                nc.sync.dma_start(sim_ext.ap()[0:1, ts(qb, 128)], sim_row[:])
                nmx = small.tile([128, 1], f32, tag="nmx")
                nc.vector.tensor_scalar_mul(nmx[:], mx[:], -1.0)
                e_sb = soft_pool.tile([128, Y], f32, tag="e")
                sum_e = small.tile([128, 1], f32, tag="sum")
                nc.scalar.activation(e_sb[:], ps_scores[:], AF.Exp,
                                     bias=nmx[:, 0:1], accum_out=sum_e[:, 0:1])
                rinv = small.tile([128, 1], f32, tag="rinv")
                nc.vector.reciprocal(rinv[:], sum_e[:])
                if split_out:
                    for h in range(2):
                        nc.vector.tensor_scalar_mul(
                            e_sb[:, ts(h, Y // 2)], e_sb[:, ts(h, Y // 2)],
                            rinv[:, 0:1])
                        nc.sync.dma_start(
                            att_ext.ap()[ts(qb, 128), ts(h, Y // 2)],
                            e_sb[:, ts(h, Y // 2)])
                else:
                    nc.vector.tensor_scalar_mul(e_sb[:], e_sb[:], rinv[:, 0:1])
                    nc.sync.dma_start(att_ext.ap()[ts(qb, 128), :], e_sb[:])

            FULL = [QC] * NCH                       # 4 x 32
            HEAD = [4, 4, 8, 16, 16, 16, 32, 32]    # ramp up ACT early
            TAIL = [32, 32, 16, 16, 8, 8, 4, 4, 4, 4]  # shrink exposed tail

            # Emission order drives the schedule: minimal t=0 path first so
            # the ACT main loop starts ASAP; the whole t=1 side hides under
            # the first (qb=0, t=0) tanh sweep (~56us of ACT work).
            load_T(wyT_ext, wyT)
            load_T(yT_ext, yT, nsplit=8)
            proj_hy(0)
            load_T(wqT_ext, wqT)
            load_T(qT_ext, qT, nsplit=4)
            small_consts()
            proj_hq(0, 0)

            scores0 = psum_sc.tile([128, Y], f32, tag="scores", name="scores0")
            sweep(0, 0, scores0, HEAD, first_sweep=True, last_sweep=False)

            # t=1 projections: fill engine idle slots under the sweep above
            proj_hy(1)
            proj_hq(1, 0)
            proj_hq(0, 1)
            proj_hq(1, 1)

            sweep(0, 1, scores0, FULL, first_sweep=False, last_sweep=True)
            softmax_block(0, scores0, split_out=False)

            scores1 = psum_sc.tile([128, Y], f32, tag="scores", name="scores1")
            sweep(1, 0, scores1, FULL, first_sweep=True, last_sweep=False)
            sweep(1, 1, scores1, TAIL, first_sweep=False, last_sweep=True)
            softmax_block(1, scores1, split_out=True)

    nc.compile()
    return nc


def _get_nc():
    global _cached
    if _cached is None:
        _cached = _build()
    return _cached


def make_in_maps(query, y, Wq_w, Wq_b, Wy_w, Wy_b, v_w, v_b):
    import ml_dtypes
    bf = ml_dtypes.bfloat16
    # host-side data marshalling: per-core batch slices, transposed + cast
    # to the kernel's internal bf16 layout (all FLOPs stay on-device)
    wqT = np.ascontiguousarray(np.asarray(Wq_w, np.float32).T.astype(bf))
    wyT = np.ascontiguousarray(np.asarray(Wy_w, np.float32).T.astype(bf))
    common = {
        "wqTv": wqT, "wyTv": wyT,
        "Wq_b": np.ascontiguousarray(Wq_b, dtype=np.float32),
        "Wy_b": np.ascontiguousarray(Wy_b, dtype=np.float32),
        "v_w": np.ascontiguousarray(v_w, dtype=np.float32),
        "v_b": np.ascontiguousarray(v_b, dtype=np.float32),
    }
    in_maps = []
    for b in range(np.asarray(query).shape[0]):
        in_maps.append({
            "qTv": np.ascontiguousarray(
                np.asarray(query[b], np.float32).T.astype(bf)),
            "yTv": np.ascontiguousarray(
                np.asarray(y[b], np.float32).T.astype(bf)),
            **common,
        })
    return in_maps


def kernel(query, y, Wq_w, Wq_b, Wy_w, Wy_b, v_w, v_b):
    from concourse.bass_utils import run_bass_kernel_spmd

    nc = _get_nc()
    in_maps = make_in_maps(query, y, Wq_w, Wq_b, Wy_w, Wy_b, v_w, v_b)
    res = run_bass_kernel_spmd(nc, in_maps, core_ids=list(range(B)))
    att = np.stack([res.results[b]["att"] for b in range(B)])
    sim = np.stack([res.results[b]["sim"] for b in range(B)])
    return att.astype(np.float32), sim.astype(np.float32)


# revision 23
# speedup vs baseline: 1.0033x; 1.0033x over previous
"""Bahdanau attention kernel for Trainium2, 8 NeuronCores, data-parallel over batch.

Reference computation (per batch b):
    hq = query @ Wq_w.T + Wq_b          # [Q, A]
    hy = y @ Wy_w.T + Wy_b              # [Y, A]
    scores[q, y] = v_w . tanh(hq[q] + hy[y]) + v_b   # [Q, Y]
    att = softmax(scores, axis=y)       # [Q, Y]
    sim[q] = max_y scores[q, y]         # [1, Q]

Shapes: B=8, Q=256, Y=512, D=1024, A=256.

Kernel strategy (one batch per core; measured 255.7us on 8 cores):
  - Inputs arrive host-transposed and bf16-cast (data marshalling in
    kernel(); all FLOPs run on-device).  The attention dim A lives on
    SBUF partitions (2 tiles of 128).
  - hqT [a, q] (f32, biases folded) and hyT [a, y] (bf16) via PE
    matmuls contracting D.
  - Per q: S[a, y] = hyT[a, y] + hqT[a, q] via DVE tensor_scalar_add
    (per-partition scalar); up to 32 q's are tanh'd per ACT instruction
    (in-place, bf16).  ACT runs >99% busy in steady state - it is the
    bottleneck engine (33.5M tanh elems @ 1 elem/lane/cycle ~ 218us).
  - Score dot: PE matmul with a sliding-window stationary (v at column
    127 of a zero [128, 256] buffer; window [127-q : 255-q] routes the
    dot to output row q), accumulating 256 MMs into one [128q, 512y]
    PSUM bank - no score transpose/gather needed.
  - t-major sweeps + chunk-size ramps overlap the second a-tile's
    projections under the first tanh sweep and shrink the exposed
    head/tail latency.
  - Softmax: DVE reduce_max -> ACT Exp(bias=-max, accum_out=sum) ->
    DVE reciprocal -> scale.  sim = max + v_b, PE-transposed to [1,128]
    for a single contiguous output DMA.
"""

import numpy as np

B, Q, Y, D, A = 8, 256, 512, 1024, 256
KT = D // 128   # k tiles in contraction dim
AT = A // 128   # a tiles
QB = Q // 128   # q blocks
YTILES = Y // 128
QTILES = Q // 128
QC = 32         # max q's per ACT chunk (S tile sizing)
NCH = 128 // QC

_cached = None


def _build():
    import concourse.bass as bass
    import concourse.tile as tile
    from concourse import bacc, mybir
    from concourse import masks

    f32 = mybir.dt.float32
    f32r = mybir.dt.float32r
    bf16 = mybir.dt.bfloat16
    ts = bass.ts
    AF = mybir.ActivationFunctionType

    nc = bacc.Bacc("TRN2", target_bir_lowering=False, debug=False)

    # inputs are host-marshalled into the exact SBUF image:
    # [128 partitions, KT*M] with k-tile k at columns [k*M, (k+1)*M)
    qT_ext = nc.dram_tensor("qTv", [128, KT * Q], bf16, kind="ExternalInput")
    yT_ext = nc.dram_tensor("yTv", [128, KT * Y], bf16, kind="ExternalInput")
    wqT_ext = nc.dram_tensor("wqTv", [128, KT * A], bf16, kind="ExternalInput")
    wyT_ext = nc.dram_tensor("wyTv", [128, KT * A], bf16, kind="ExternalInput")
    wqb_ext = nc.dram_tensor("Wq_b", [A], f32, kind="ExternalInput")
    wyb_ext = nc.dram_tensor("Wy_b", [A], f32, kind="ExternalInput")
    v_ext = nc.dram_tensor("v_w", [1, A], f32, kind="ExternalInput")
    vb_ext = nc.dram_tensor("v_b", [1], f32, kind="ExternalInput")
    att_ext = nc.dram_tensor("att", [Q, Y], f32, kind="ExternalOutput")
    sim_ext = nc.dram_tensor("sim", [1, Q], f32, kind="ExternalOutput")

    with tile.TileContext(nc) as tc:
        from contextlib import ExitStack
        ctx = ExitStack()
        with ctx:
            consts = ctx.enter_context(tc.tile_pool(name="consts", bufs=1))
            tr_sb = ctx.enter_context(tc.tile_pool(name="tr_sb", bufs=1))
            s_pool = ctx.enter_context(tc.tile_pool(name="s", bufs=4))
            soft_pool = ctx.enter_context(tc.tile_pool(name="soft", bufs=2))
            small = ctx.enter_context(tc.tile_pool(name="small", bufs=2))
            psum_proj = ctx.enter_context(
                tc.tile_pool(name="ps_proj", bufs=1, space="PSUM"))
            psum_sc = ctx.enter_context(
                tc.tile_pool(name="ps_sc", bufs=2, space="PSUM"))

            # ---- constants ----
            ones_row = consts.tile([1, 128], f32)
            nc.gpsimd.memset(ones_row[:], 1.0)

            # per-partition vectors (DMAs issued later, after the big
            # input loads, so they don't delay them on the sync queue)
            bq_sb = [consts.tile([128, 1], f32, name=f"bq{t}") for t in range(AT)]
            by_sb = [consts.tile([128, 1], f32, name=f"by{t}") for t in range(AT)]
            v_sb = [consts.tile([128, 1], f32, name=f"v{t}") for t in range(AT)]
            vb_sb = consts.tile([1, 1], f32)
            cb = [consts.tile([128, 1], f32, name=f"cb{t}") for t in range(AT)]
            vb_bc = consts.tile([128, 1], f32)
            zv = [consts.tile([128, 256], bf16, name=f"zv{t}") for t in range(AT)]

            def small_consts():
                for t in range(AT):
                    nc.sync.dma_start(bq_sb[t][:], wqb_ext.ap()[ts(t, 128)].unsqueeze(1))
                    nc.sync.dma_start(by_sb[t][:], wyb_ext.ap()[ts(t, 128)].unsqueeze(1))
                    nc.sync.dma_start(v_sb[t][:], v_ext.ap()[0, ts(t, 128)].unsqueeze(1))
                nc.sync.dma_start(vb_sb[:], vb_ext.ap().unsqueeze(0))
                for t in range(AT):
                    nc.vector.tensor_add(cb[t][:], bq_sb[t][:], by_sb[t][:])
                ps_vb = psum_proj.tile([128, 1], f32, tag="hy")
                nc.tensor.matmul(ps_vb[:], ones_row[:], vb_sb[:], start=True, stop=True)
                nc.vector.tensor_copy(vb_bc[:], ps_vb[:])
                for t in range(AT):
                    nc.gpsimd.memset(zv[t][:], 0.0)
                    nc.vector.tensor_copy(zv[t][:, 127:128], v_sb[t][:])

            # ---- load transposed bf16 inputs (host-marshalled) ----
            ident = consts.tile([128, 128], bf16)
            masks.make_identity(nc, ident[:])
            ident_f32 = consts.tile([128, 128], f32)
            masks.make_identity(nc, ident_f32[:])

            # SBUF layout: block k at columns [k*M, (k+1)*M)
            qT = tr_sb.tile([128, KT * Q], bf16, name="qT")
            yT = tr_sb.tile([128, KT * Y], bf16, name="yT")
            wqT = tr_sb.tile([128, KT * A], bf16, name="wqT")
            wyT = tr_sb.tile([128, KT * A], bf16, name="wyT")

            def load_T(ext, dst):
                # DRAM image is already the SBUF layout: one fully
                # contiguous DMA (8KB/partition max packets).
                nc.sync.dma_start(dst[:], ext.ap())

            # PE warm-up: ~20 dummy matmuls flip the HAM clock gate to
            # 2.4 GHz before the projection chains run; consumed via a
            # DRAM scratch store so DCE keeps them.
            warm_dram = nc.dram_tensor("warm_scratch", [128, 128], f32)
            ps_warm = psum_sc.tile([128, 128], f32, tag="warm", bufs=1)
            NWARM = 44
            for w in range(NWARM):
                nc.tensor.matmul(ps_warm[:], ident[:], ident[:],
                                 start=(w == 0), stop=(w == NWARM - 1))
            warm_sb = small.tile([128, 128], f32, tag="warm")
            nc.vector.tensor_copy(warm_sb[:], ps_warm[:])
            nc.sync.dma_start(warm_dram.ap(), warm_sb[:])

            hyT = [tr_sb.tile([128, Y], bf16, name=f"hyT{t}") for t in range(AT)]
            hqT = [tr_sb.tile([128, Q], f32, name=f"hqT{t}") for t in range(AT)]

            def proj_hy(t):
                ps = psum_proj.tile([128, Y], f32, tag="hy")
                for k in range(KT):
                    nc.tensor.matmul(
                        ps[:], wyT[:, k * A + t * 128:k * A + t * 128 + 128],
                        yT[:, ts(k, Y)],
                        start=(k == 0), stop=(k == KT - 1))
                nc.vector.tensor_copy(hyT[t][:], ps[:])

            def proj_hq(t, h, c0=0, c1=128):
                # one q-column range of half h (separate accumulation group)
                ps = psum_proj.tile([128, 128], f32, tag="hq")
                n = c1 - c0
                for k in range(KT):
                    nc.tensor.matmul(
                        ps[:, 0:n],
                        wqT[:, k * A + t * 128:k * A + t * 128 + 128],
                        qT[:, k * Q + h * 128 + c0:k * Q + h * 128 + c1],
                        start=(k == 0), stop=(k == KT - 1))
                nc.vector.tensor_scalar_add(
                    hqT[t][:, h * 128 + c0:h * 128 + c1], ps[:, 0:n],
                    cb[t][:, 0:1])

            # ---- main loop pieces ----
            def sweep(qb, t, ps_scores, chunks, first_sweep, last_sweep):
                # one (q-block, a-tile) pass: S = hy + hq[q] (DVE), tanh
                # (ACT, in-place), score-dot MMs into ps_scores.
                q0 = 0
                for ci, qc in enumerate(chunks):
                    S = s_pool.tile([128, QC * Y], bf16, tag="S")
                    for j in range(qc):
                        q = qb * 128 + q0 + j
                        nc.vector.tensor_scalar_add(
                            S[:, ts(j, Y)], hyT[t][:], hqT[t][:, q:q + 1])
                    nc.scalar.activation(S[:, 0:qc * Y], S[:, 0:qc * Y], AF.Tanh)
                    for j in range(qc):
                        ql = q0 + j
                        first = (first_sweep and ci == 0 and j == 0)
                        last = (last_sweep and ci == len(chunks) - 1
                                and j == qc - 1)
                        nc.tensor.matmul(
                            ps_scores[:],
                            zv[t][:, 127 - ql:255 - ql],
                            S[:, ts(j, Y)],
                            start=first, stop=last)
                    q0 += qc

            def softmax_block(qb, ps_scores, split_out):
                mx = small.tile([128, 1], f32, tag="mx")
                nc.vector.reduce_max(mx[:], ps_scores[:], axis=mybir.AxisListType.X)
                sim_sb = small.tile([128, 1], f32, tag="sim")
                nc.vector.tensor_add(sim_sb[:], mx[:], vb_bc[:])
                ps_simT = psum_proj.tile([1, 128], f32, tag="hq", name=f"psimT{qb}")
                nc.tensor.transpose(ps_simT[:], sim_sb[:], ident_f32[:])
                sim_row = small.tile([1, 128], f32, tag="simrow")
                nc.vector.tensor_copy(sim_row[:], ps_simT[:])
                nc.sync.dma_start(sim_ext.ap()[0:1, ts(qb, 128)], sim_row[:])
                nmx = small.tile([128, 1], f32, tag="nmx")
                nc.vector.tensor_scalar_mul(nmx[:], mx[:], -1.0)
                e_sb = soft_pool.tile([128, Y], f32, tag="e")
                sum_e = small.tile([128, 1], f32, tag="sum")
                nc.scalar.activation(e_sb[:], ps_scores[:], AF.Exp,
                                     bias=nmx[:, 0:1], accum_out=sum_e[:, 0:1])
                rinv = small.tile([128, 1], f32, tag="rinv")
                nc.vector.reciprocal(rinv[:], sum_e[:])
                if split_out:
                    for h in range(2):
                        nc.vector.tensor_scalar_mul(
                            e_sb[:, ts(h, Y // 2)], e_sb[:, ts(h, Y // 2)],
                            rinv[:, 0:1])
                        nc.sync.dma_start(
                            att_ext.ap()[ts(qb, 128), ts(h, Y // 2)],
                            e_sb[:, ts(h, Y // 2)])
                else:
                    nc.vector.tensor_scalar_mul(e_sb[:], e_sb[:], rinv[:, 0:1])
                    nc.sync.dma_start(att_ext.ap()[ts(qb, 128), :], e_sb[:])

            FULL = [QC] * NCH                       # 4 x 32
            HEAD = [4, 4, 8, 16, 16, 16, 32, 32]    # ramp up ACT early
            TAIL = [32, 32, 16, 16, 8, 8, 4, 4, 4, 4]  # shrink exposed tail

            # Emission order drives the schedule: minimal t=0 path first so
            # the ACT main loop starts ASAP; the whole t=1 side hides under
            # the first (qb=0, t=0) tanh sweep (~56us of ACT work).
            load_T(wyT_ext, wyT)
            load_T(yT_ext, yT)
            proj_hy(0)
            load_T(wqT_ext, wqT)
            load_T(qT_ext, qT)
            small_consts()
            proj_hq(0, 0)

            scores0 = psum_sc.tile([128, Y], f32, tag="scores", name="scores0")
            sweep(0, 0, scores0, HEAD, first_sweep=True, last_sweep=False)

            # t=1 projections: fill engine idle slots under the sweep above
            proj_hy(1)
            proj_hq(1, 0)
            proj_hq(0, 1)
            proj_hq(1, 1)

            sweep(0, 1, scores0, FULL, first_sweep=False, last_sweep=True)
            softmax_block(0, scores0, split_out=False)

            scores1 = psum_sc.tile([128, Y], f32, tag="scores", name="scores1")
            sweep(1, 0, scores1, FULL, first_sweep=True, last_sweep=False)
            sweep(1, 1, scores1, TAIL, first_sweep=False, last_sweep=True)
            softmax_block(1, scores1, split_out=True)

    nc.compile()
    return nc


def _get_nc():
    global _cached
    if _cached is None:
        _cached = _build()
    return _cached


def make_in_maps(query, y, Wq_w, Wq_b, Wy_w, Wy_b, v_w, v_b):
    import ml_dtypes
    bf = ml_dtypes.bfloat16
    # host-side data marshalling: per-core batch slices, transposed + cast
    # to the kernel's internal bf16 layout (all FLOPs stay on-device)
    def sbuf_image(mat):
        # [M, D] f32 -> transposed bf16 SBUF image [128, KT*M]
        T = np.asarray(mat, np.float32).T.astype(bf)      # [D, M]
        M = T.shape[1]
        return np.ascontiguousarray(
            T.reshape(KT, 128, M).transpose(1, 0, 2).reshape(128, KT * M))

    wqT = sbuf_image(Wq_w)
    wyT = sbuf_image(Wy_w)
    common = {
        "wqTv": wqT, "wyTv": wyT,
        "Wq_b": np.ascontiguousarray(Wq_b, dtype=np.float32),
        "Wy_b": np.ascontiguousarray(Wy_b, dtype=np.float32),
        "v_w": np.ascontiguousarray(v_w, dtype=np.float32),
        "v_b": np.ascontiguousarray(v_b, dtype=np.float32),
    }
    in_maps = []
    for b in range(np.asarray(query).shape[0]):
        in_maps.append({
            "qTv": sbuf_image(query[b]),
            "yTv": sbuf_image(y[b]),
            **common,
        })
    return in_maps


def kernel(query, y, Wq_w, Wq_b, Wy_w, Wy_b, v_w, v_b):
    from concourse.bass_utils import run_bass_kernel_spmd

    nc = _get_nc()
    in_maps = make_in_maps(query, y, Wq_w, Wq_b, Wy_w, Wy_b, v_w, v_b)
    res = run_bass_kernel_spmd(nc, in_maps, core_ids=list(range(B)))
    att = np.stack([res.results[b]["att"] for b in range(B)])
    sim = np.stack([res.results[b]["sim"] for b in range(B)])
    return att.astype(np.float32), sim.astype(np.float32)


# revision 25
# speedup vs baseline: 1.0066x; 1.0034x over previous
"""Bahdanau attention kernel for Trainium2, 8 NeuronCores, data-parallel over batch.

Reference computation (per batch b):
    hq = query @ Wq_w.T + Wq_b          # [Q, A]
    hy = y @ Wy_w.T + Wy_b              # [Y, A]
    scores[q, y] = v_w . tanh(hq[q] + hy[y]) + v_b   # [Q, Y]
    att = softmax(scores, axis=y)       # [Q, Y]
    sim[q] = max_y scores[q, y]         # [1, Q]

Shapes: B=8, Q=256, Y=512, D=1024, A=256.

Kernel strategy (one batch per core; measured 255.6us on 8 cores):
  - Inputs arrive host-transposed and bf16-cast (data marshalling in
    kernel(); all FLOPs run on-device).  The attention dim A lives on
    SBUF partitions (2 tiles of 128).
  - hqT [a, q] (f32, biases folded) and hyT [a, y] (bf16) via PE
    matmuls contracting D.
  - Per q: S[a, y] = hyT[a, y] + hqT[a, q] via DVE tensor_scalar_add
    (per-partition scalar); up to 32 q's are tanh'd per ACT instruction
    (in-place, bf16).  ACT runs >99% busy in steady state - it is the
    bottleneck engine (33.5M tanh elems @ 1 elem/lane/cycle ~ 218us).
  - Score dot: PE matmul with a sliding-window stationary (v at column
    127 of a zero [128, 256] buffer; window [127-q : 255-q] routes the
    dot to output row q), accumulating 256 MMs into one [128q, 512y]
    PSUM bank - no score transpose/gather needed.
  - t-major sweeps + chunk-size ramps overlap the second a-tile's
    projections under the first tanh sweep and shrink the exposed
    head/tail latency.
  - Softmax: DVE reduce_max -> ACT Exp(bias=-max, accum_out=sum) ->
    DVE reciprocal -> scale.  sim = max + v_b, PE-transposed to [1,128]
    for a single contiguous output DMA.
"""

import numpy as np

B, Q, Y, D, A = 8, 256, 512, 1024, 256
KT = D // 128   # k tiles in contraction dim
AT = A // 128   # a tiles
QB = Q // 128   # q blocks
YTILES = Y // 128
QTILES = Q // 128
QC = 32         # max q's per ACT chunk (S tile sizing)
NCH = 128 // QC

_cached = None


def _build():
    import concourse.bass as bass
    import concourse.tile as tile
    from concourse import bacc, mybir
    from concourse import masks

    f32 = mybir.dt.float32
    f32r = mybir.dt.float32r
    bf16 = mybir.dt.bfloat16
    ts = bass.ts
    AF = mybir.ActivationFunctionType

    nc = bacc.Bacc("TRN2", target_bir_lowering=False, debug=False)

    # inputs are host-marshalled into the exact SBUF image:
    # [128 partitions, KT*M] with k-tile k at columns [k*M, (k+1)*M)
    qT_ext = nc.dram_tensor("qTv", [128, KT * Q], bf16, kind="ExternalInput")
    yT_ext = nc.dram_tensor("yTv", [128, KT * Y], bf16, kind="ExternalInput")
    wqT_ext = nc.dram_tensor("wqTv", [128, KT * A], bf16, kind="ExternalInput")
    wyT_ext = nc.dram_tensor("wyTv", [128, KT * A], bf16, kind="ExternalInput")
    wqb_ext = nc.dram_tensor("Wq_b", [A], f32, kind="ExternalInput")
    wyb_ext = nc.dram_tensor("Wy_b", [A], f32, kind="ExternalInput")
    v_ext = nc.dram_tensor("v_w", [1, A], f32, kind="ExternalInput")
    vb_ext = nc.dram_tensor("v_b", [1], f32, kind="ExternalInput")
    att_ext = nc.dram_tensor("att", [Q, Y], f32, kind="ExternalOutput")
    sim_ext = nc.dram_tensor("sim", [1, Q], f32, kind="ExternalOutput")

    with tile.TileContext(nc) as tc:
        from contextlib import ExitStack
        ctx = ExitStack()
        with ctx:
            consts = ctx.enter_context(tc.tile_pool(name="consts", bufs=1))
            tr_sb = ctx.enter_context(tc.tile_pool(name="tr_sb", bufs=1))
            s_pool = ctx.enter_context(tc.tile_pool(name="s", bufs=4))
            soft_pool = ctx.enter_context(tc.tile_pool(name="soft", bufs=2))
            small = ctx.enter_context(tc.tile_pool(name="small", bufs=2))
            psum_proj = ctx.enter_context(
                tc.tile_pool(name="ps_proj", bufs=1, space="PSUM"))
            psum_sc = ctx.enter_context(
                tc.tile_pool(name="ps_sc", bufs=2, space="PSUM"))

            # ---- constants ----
            ones_row = consts.tile([1, 128], f32)
            nc.gpsimd.memset(ones_row[:], 1.0)

            # per-partition vectors (DMAs issued later, after the big
            # input loads, so they don't delay them on the sync queue)
            bq_sb = [consts.tile([128, 1], f32, name=f"bq{t}") for t in range(AT)]
            by_sb = [consts.tile([128, 1], f32, name=f"by{t}") for t in range(AT)]
            v_sb = [consts.tile([128, 1], f32, name=f"v{t}") for t in range(AT)]
            vb_sb = consts.tile([1, 1], f32)
            cb = [consts.tile([128, 1], f32, name=f"cb{t}") for t in range(AT)]
            vb_bc = consts.tile([128, 1], f32)
            zv = [consts.tile([128, 256], bf16, name=f"zv{t}") for t in range(AT)]

            def small_consts():
                for t in range(AT):
                    nc.sync.dma_start(bq_sb[t][:], wqb_ext.ap()[ts(t, 128)].unsqueeze(1))
                    nc.sync.dma_start(by_sb[t][:], wyb_ext.ap()[ts(t, 128)].unsqueeze(1))
                    nc.sync.dma_start(v_sb[t][:], v_ext.ap()[0, ts(t, 128)].unsqueeze(1))
                nc.sync.dma_start(vb_sb[:], vb_ext.ap().unsqueeze(0))
                for t in range(AT):
                    nc.vector.tensor_add(cb[t][:], bq_sb[t][:], by_sb[t][:])
                ps_vb = psum_proj.tile([128, 1], f32, tag="hy")
                nc.tensor.matmul(ps_vb[:], ones_row[:], vb_sb[:], start=True, stop=True)
                nc.vector.tensor_copy(vb_bc[:], ps_vb[:])
                for t in range(AT):
                    nc.gpsimd.memset(zv[t][:], 0.0)
                    nc.vector.tensor_copy(zv[t][:, 127:128], v_sb[t][:])

            # ---- load transposed bf16 inputs (host-marshalled) ----
            ident = consts.tile([128, 128], bf16)
            masks.make_identity(nc, ident[:])
            ident_f32 = consts.tile([128, 128], f32)
            masks.make_identity(nc, ident_f32[:])

            # SBUF layout: block k at columns [k*M, (k+1)*M)
            qT = tr_sb.tile([128, KT * Q], bf16, name="qT")
            yT = tr_sb.tile([128, KT * Y], bf16, name="yT")
            wqT = tr_sb.tile([128, KT * A], bf16, name="wqT")
            wyT = tr_sb.tile([128, KT * A], bf16, name="wyT")

            def load_T(ext, dst):
                # DRAM image is already the SBUF layout: one fully
                # contiguous DMA (8KB/partition max packets).
                nc.sync.dma_start(dst[:], ext.ap())

            # PE warm-up: ~20 dummy matmuls flip the HAM clock gate to
            # 2.4 GHz before the projection chains run; consumed via a
            # DRAM scratch store so DCE keeps them.
            warm_dram = nc.dram_tensor("warm_scratch", [128, 128], f32)
            ps_warm = psum_sc.tile([128, 128], f32, tag="warm", bufs=1)
            NWARM = 110
            for w in range(NWARM):
                nc.tensor.matmul(ps_warm[:], ident[:], ident[:],
                                 start=(w == 0), stop=(w == NWARM - 1))
            warm_sb = small.tile([128, 128], f32, tag="warm")
            nc.vector.tensor_copy(warm_sb[:], ps_warm[:])
            nc.sync.dma_start(warm_dram.ap(), warm_sb[:])

            hyT = [tr_sb.tile([128, Y], bf16, name=f"hyT{t}") for t in range(AT)]
            hqT = [tr_sb.tile([128, Q], f32, name=f"hqT{t}") for t in range(AT)]

            def proj_hy(t):
                ps = psum_proj.tile([128, Y], f32, tag="hy")
                for k in range(KT):
                    nc.tensor.matmul(
                        ps[:], wyT[:, k * A + t * 128:k * A + t * 128 + 128],
                        yT[:, ts(k, Y)],
                        start=(k == 0), stop=(k == KT - 1))
                nc.vector.tensor_copy(hyT[t][:], ps[:])

            def proj_hq(t, h, c0=0, c1=128):
                # one q-column range of half h (separate accumulation group)
                ps = psum_proj.tile([128, 128], f32, tag="hq")
                n = c1 - c0
                for k in range(KT):
                    nc.tensor.matmul(
                        ps[:, 0:n],
                        wqT[:, k * A + t * 128:k * A + t * 128 + 128],
                        qT[:, k * Q + h * 128 + c0:k * Q + h * 128 + c1],
                        start=(k == 0), stop=(k == KT - 1))
                nc.vector.tensor_scalar_add(
                    hqT[t][:, h * 128 + c0:h * 128 + c1], ps[:, 0:n],
                    cb[t][:, 0:1])

            # ---- main loop pieces ----
            def sweep(qb, t, ps_scores, chunks, first_sweep, last_sweep):
                # one (q-block, a-tile) pass: S = hy + hq[q] (DVE), tanh
                # (ACT, in-place), score-dot MMs into ps_scores.
                q0 = 0
                for ci, qc in enumerate(chunks):
                    S = s_pool.tile([128, QC * Y], bf16, tag="S")
                    for j in range(qc):
                        q = qb * 128 + q0 + j
                        nc.vector.tensor_scalar_add(
                            S[:, ts(j, Y)], hyT[t][:], hqT[t][:, q:q + 1])
                    nc.scalar.activation(S[:, 0:qc * Y], S[:, 0:qc * Y], AF.Tanh)
                    for j in range(qc):
                        ql = q0 + j
                        first = (first_sweep and ci == 0 and j == 0)
                        last = (last_sweep and ci == len(chunks) - 1
                                and j == qc - 1)
                        nc.tensor.matmul(
                            ps_scores[:],
                            zv[t][:, 127 - ql:255 - ql],
                            S[:, ts(j, Y)],
                            start=first, stop=last)
                    q0 += qc

            def softmax_block(qb, ps_scores, split_out):
                mx = small.tile([128, 1], f32, tag="mx")
                nc.vector.reduce_max(mx[:], ps_scores[:], axis=mybir.AxisListType.X)
                sim_sb = small.tile([128, 1], f32, tag="sim")
                nc.vector.tensor_add(sim_sb[:], mx[:], vb_bc[:])
                ps_simT = psum_proj.tile([1, 128], f32, tag="hq", name=f"psimT{qb}")
                nc.tensor.transpose(ps_simT[:], sim_sb[:], ident_f32[:])
                sim_row = small.tile([1, 128], f32, tag="simrow")
                nc.vector.tensor_copy(sim_row[:], ps_simT[:])
                nc.sync.dma_start(sim_ext.ap()[0:1, ts(qb, 128)], sim_row[:])
                nmx = small.tile([128, 1], f32, tag="nmx")
                nc.vector.tensor_scalar_mul(nmx[:], mx[:], -1.0)
                e_sb = soft_pool.tile([128, Y], f32, tag="e")
                sum_e = small.tile([128, 1], f32, tag="sum")
                nc.scalar.activation(e_sb[:], ps_scores[:], AF.Exp,
                                     bias=nmx[:, 0:1], accum_out=sum_e[:, 0:1])
                rinv = small.tile([128, 1], f32, tag="rinv")
                nc.vector.reciprocal(rinv[:], sum_e[:])
                if split_out:
                    for h in range(2):
                        nc.vector.tensor_scalar_mul(
                            e_sb[:, ts(h, Y // 2)], e_sb[:, ts(h, Y // 2)],
                            rinv[:, 0:1])
                        nc.sync.dma_start(
                            att_ext.ap()[ts(qb, 128), ts(h, Y // 2)],
                            e_sb[:, ts(h, Y // 2)])
                else:
                    nc.vector.tensor_scalar_mul(e_sb[:], e_sb[:], rinv[:, 0:1])
                    nc.sync.dma_start(att_ext.ap()[ts(qb, 128), :], e_sb[:])

            FULL = [QC] * NCH                       # 4 x 32
            HEAD = [4, 4, 8, 16, 16, 16, 32, 32]    # ramp up ACT early
            TAIL = [32, 32, 16, 16, 8, 8, 4, 4, 4, 4]  # shrink exposed tail

            # Emission order drives the schedule: minimal t=0 path first so
            # the ACT main loop starts ASAP; the whole t=1 side hides under
            # the first (qb=0, t=0) tanh sweep (~56us of ACT work).
            load_T(wyT_ext, wyT)
            load_T(yT_ext, yT)
            proj_hy(0)
            load_T(wqT_ext, wqT)
            load_T(qT_ext, qT)
            small_consts()
            proj_hq(0, 0)

            scores0 = psum_sc.tile([128, Y], f32, tag="scores", name="scores0")
            sweep(0, 0, scores0, HEAD, first_sweep=True, last_sweep=False)

            # t=1 projections: fill engine idle slots under the sweep above
            proj_hy(1)
            proj_hq(1, 0)
            proj_hq(0, 1)
            proj_hq(1, 1)

            sweep(0, 1, scores0, FULL, first_sweep=False, last_sweep=True)
            softmax_block(0, scores0, split_out=False)

            scores1 = psum_sc.tile([128, Y], f32, tag="scores", name="scores1")
            sweep(1, 0, scores1, FULL, first_sweep=True, last_sweep=False)
            sweep(1, 1, scores1, TAIL, first_sweep=False, last_sweep=True)
            softmax_block(1, scores1, split_out=True)

    nc.compile()
    return nc


def _get_nc():
    global _cached
    if _cached is None:
        _cached = _build()
    return _cached


def make_in_maps(query, y, Wq_w, Wq_b, Wy_w, Wy_b, v_w, v_b):
    import ml_dtypes
    bf = ml_dtypes.bfloat16
    # host-side data marshalling: per-core batch slices, transposed + cast
    # to the kernel's internal bf16 layout (all FLOPs stay on-device)
    def sbuf_image(mat):
        # [M, D] f32 -> transposed bf16 SBUF image [128, KT*M]
        T = np.asarray(mat, np.float32).T.astype(bf)      # [D, M]
        M = T.shape[1]
        return np.ascontiguousarray(
            T.reshape(KT, 128, M).transpose(1, 0, 2).reshape(128, KT * M))

    wqT = sbuf_image(Wq_w)
    wyT = sbuf_image(Wy_w)
    common = {
        "wqTv": wqT, "wyTv": wyT,
        "Wq_b": np.ascontiguousarray(Wq_b, dtype=np.float32),
        "Wy_b": np.ascontiguousarray(Wy_b, dtype=np.float32),
        "v_w": np.ascontiguousarray(v_w, dtype=np.float32),
        "v_b": np.ascontiguousarray(v_b, dtype=np.float32),
    }
    in_maps = []
    for b in range(np.asarray(query).shape[0]):
        in_maps.append({
            "qTv": sbuf_image(query[b]),
            "yTv": sbuf_image(y[b]),
            **common,
        })
    return in_maps


def kernel(query, y, Wq_w, Wq_b, Wy_w, Wy_b, v_w, v_b):
    from concourse.bass_utils import run_bass_kernel_spmd

    nc = _get_nc()
    in_maps = make_in_maps(query, y, Wq_w, Wq_b, Wy_w, Wy_b, v_w, v_b)
    res = run_bass_kernel_spmd(nc, in_maps, core_ids=list(range(B)))
    att = np.stack([res.results[b]["att"] for b in range(B)])
    sim = np.stack([res.results[b]["sim"] for b in range(B)])
    return att.astype(np.float32), sim.astype(np.float32)
